# revision 1
# baseline (speedup 1.0000x reference)
"""DoRA embedding kernel for 8 Trainium2 NeuronCores.

Math (reference):
    C = E + s * A @ B                  # [V, D]
    n = max(||C||_col, 1e-8)           # [D]
    out = (C / n * mag)[token_ids]     # [B, S, D]

Strategy: shard D=768 columns across 8 cores (96 cols each), no collectives.

Pass 1 (norms) — self-Gram on PE, fp8:
    Stream vocab-major tiles T = [256*E_cols | 256*A] (fp8, [128v, 112], four
    vocab rows packed per 512B table row for full-rate DMA) through
    matmul(T^T T), accumulating G = [[E'E', E'A'],[A'E', A'A']] in one PSUM
    bank (396 matmuls, one accumulation group).  Then
        sumsq*65536 = diag(E'E') + sum_r (sB (.) (2M^T + A'A' sB))
    via one DVE diag-mask reduce, one tiny fused matmul, and a
    partition-reduce matmul, all [16,96]-sized.  rsqrt via DVE reciprocal +
    one Newton step, in [1,96] orientation; broadcast to [128,96] by PE.

Pass 2 (lookup) — parity-bucketed 256B-row gathers, token-major, bf16:
    Host splits tokens into even/odd vocab-id buckets so the pair index
    id//2 fits int16 and each token gathers only its 256B half-row
    [E_row | A_row | pad] (elem_step=512B over the pair table).
    Per chunk of 1024 tokens: PE transposes the A columns ([128,16] ->
    [16,128] bf16), one ACT copy to SBUF, then per 128-token slot
    A^T @ sB + identity-add of E into PSUM; DVE applies the broadcast scale
    and casts bf16; DMA out [128, 768] per chunk.  Host inverts the
    permutation and upcasts.
"""

import sys
from contextlib import ExitStack

import numpy as np

for _p in ("/opt/trn_rl_repo",):
    if _p not in sys.path:
        sys.path.append(_p)

import ml_dtypes
import concourse.bass as bass
import concourse.bacc as bacc
import concourse.tile as tile
from concourse import mybir, bass_utils

F32 = mybir.dt.float32
BF16 = mybir.dt.bfloat16
FP8 = mybir.dt.float8e4
I16 = mybir.dt.int16
ALU = mybir.AluOpType
ACTF = mybir.ActivationFunctionType

V, D, R = 50257, 768, 16
SCALING = 32.0 / 16.0
N_CORES = 8
CPC = D // N_CORES          # columns per core = 96
W = CPC + R                 # 112 packed feature width
EF8 = 256.0                 # fp8 pre-scale for E and A in pass 1
VP = 50688                  # vocab padded: 99 groups of 512 rows
NG = VP // 512              # 99 row-groups (4 matmuls each)
GL = 9                      # groups per DMA macro-load (99 = 11 * 9)
PAIRS = ((V + 1) // 2 + 127) // 128 * 128  # 25216 pair rows
NTOK = 8 * 2048
CHUNK = 1024                # tokens per dma_gather (num_idxs>1024 crashes Q7)
NS = CHUNK // 128           # 8 slots per chunk
HS = NS // 2                # half-chunk slots (PSUM bank sizing)


def _apply_drain_patch():
    """walrus in this container rejects >1 sem-wait on the Tile tail drain
    ("Too many sync wait commands"); split the waits across chained drains."""
    import concourse.tile as _tile_mod
    if getattr(_tile_mod.TileContext, "_drain_patch_applied", False):
        return

    def _drain_and_barrier(self, tick_clock, wait_clock):
        from concourse.tile import ScopedClock

        nc = self.nc
        drain_inst = nc.sync.drain()
        wait_clock.add_sem_waits(
            drain_inst.ins, ScopedClock({None: tick_clock.global_clock})
        )
        si = drain_inst.ins.sync_info
        if si is not None and si.on_wait and len(si.on_wait) > 1:
            waits = list(si.on_wait)
            del si.on_wait[1:]
            for w in waits[1:]:
                extra = nc.sync.drain()
                esi = extra.ins.sync_info
                if esi is None:
                    extra.ins.sync_info = mybir.SyncInfo(on_wait=[w], on_update=[])
                else:
                    esi.on_wait.append(w)
        nc.all_engine_barrier()
        assert self.sems is not None
        popped = nc._tile_sem_poison_stack.pop()
        assert popped is self._sem_poison
        nc.clear_and_free_semaphores(list(self.sems.allocated().values()))
        nc.all_engine_barrier()

    _tile_mod.TileContext._drain_and_barrier = _drain_and_barrier
    _tile_mod.TileContext._drain_patch_applied = True


def _bcast2(ap_2d_pair, n):
    """[P, F] AP -> [P, n, F] AP broadcasting along a 0-stride middle dim."""
    return bass.AP(
        tensor=ap_2d_pair.tensor,
        offset=ap_2d_pair.offset,
        ap=[list(ap_2d_pair.ap[0]), [0, n], list(ap_2d_pair.ap[1])],
    )


_CACHED = {}
NREP = 1
VARIANT = "full"


def _build(nch):
    key = (NREP, VARIANT, nch, tuple(_build.chunk_parity))
    if key in _CACHED:
        return _CACHED[key]
    _apply_drain_patch()

    nc = bacc.Bacc("TRN2", target_bir_lowering=False, debug=False)
    d_p1t = nc.dram_tensor("p1t", [VP // 4, 512], FP8, kind="ExternalInput").ap()
    d_gtab = nc.dram_tensor("gtab", [PAIRS, 256], BF16, kind="ExternalInput").ap()
    d_sb = nc.dram_tensor("sb", [R, CPC], F32, kind="ExternalInput").ap()
    d_sbbf = nc.dram_tensor("sbbf", [R, CPC], BF16, kind="ExternalInput").ap()
    d_mag = nc.dram_tensor("magT", [1, CPC], F32, kind="ExternalInput").ap()
    d_idf = nc.dram_tensor("identf", [128, 128], F32, kind="ExternalInput").ap()
    d_idb = nc.dram_tensor("idb", [128, 128], BF16, kind="ExternalInput").ap()
    d_i96 = nc.dram_tensor("i96", [CPC, CPC], F32, kind="ExternalInput").ap()
    d_twoi = nc.dram_tensor("twoi16", [R, R], F32, kind="ExternalInput").ap()
    d_ones16 = nc.dram_tensor("ones16", [R, 1], F32, kind="ExternalInput").ap()
    d_ones = nc.dram_tensor("ones1", [1, 128], F32, kind="ExternalInput").ap()
    d_pidx = nc.dram_tensor("pidx", [128, nch * (CHUNK // 16)], I16,
                            kind="ExternalInput").ap()
    # token-major output: token j=c*1024+s*128+p at row c*128+p, cols s*96:+96
    d_out = nc.dram_tensor("out", [nch * 128, NS * CPC], BF16,
                           kind="ExternalOutput").ap()

    with tile.TileContext(nc) as tc, ExitStack() as ctx:
        const = ctx.enter_context(tc.tile_pool(name="const", bufs=1))

        sb_sb = const.tile([R, CPC], F32)
        nc.sync.dma_start(out=sb_sb, in_=d_sb)
        sbbf_sb = const.tile([R, CPC], BF16)
        nc.sync.dma_start(out=sbbf_sb, in_=d_sbbf)
        mag_sb = const.tile([1, CPC], F32)
        nc.sync.dma_start(out=mag_sb, in_=d_mag)
        idf_sb = const.tile([128, 128], F32)
        nc.sync.dma_start(out=idf_sb, in_=d_idf)
        idb_sb = const.tile([128, 128], BF16)
        nc.sync.dma_start(out=idb_sb, in_=d_idb)
        i96_sb = const.tile([CPC, CPC], F32)
        nc.sync.dma_start(out=i96_sb, in_=d_i96)
        twoi_sb = const.tile([R, R], F32)
        nc.sync.dma_start(out=twoi_sb, in_=d_twoi)
        ones16_sb = const.tile([R, 1], F32)
        nc.sync.dma_start(out=ones16_sb, in_=d_ones16)
        ones_sb = const.tile([1, 128], F32)
        nc.sync.dma_start(out=ones_sb, in_=d_ones)
        pidx_sb = const.tile([128, nch * (CHUNK // 16)], I16)
        nc.sync.dma_start(out=pidx_sb, in_=d_pidx)

        def _dummy_out(rep_pool, src=None):
            outt0 = rep_pool.tile([CPC, 8], BF16)
            if src is None:
                nc.vector.memset(outt0, 0.0)
            else:
                nc.vector.memset(outt0, 0.0)
                nc.vector.tensor_copy(out=outt0[:, 0:1], in_=src)
            nc.sync.dma_start(out=d_out[0:CPC, 0:8], in_=outt0)

        def _emit(chunk_parity, rep_pool, p1l, p1ps, sc, scps, p2g, p2tp,
                  p2s, p2ps, p2o):
            if VARIANT == "nop":
                _dummy_out(rep_pool)
                return

            # ---- pass 2 gathers: 256B rows, token-major (issue first) ----
            gt = []
            n_gather = {"p2g1": 1}.get(VARIANT, nch)
            if VARIANT not in ("p1", "p1mm", "p1dma", "p1z", "p1a", "p1b"):
                for c in range(n_gather):
                    g = p2g.tile([128, NS, 128], BF16)
                    if VARIANT == "p2n":
                        nc.vector.memset(g, 0.25)
                    else:
                        off = 128 * chunk_parity[c]
                        nc.gpsimd.dma_gather(
                            g[:, :, :],
                            d_gtab[:, off : off + 128],
                            pidx_sb[:, c * (CHUNK // 16) : (c + 1) * (CHUNK // 16)],
                            num_idxs=CHUNK,
                            num_idxs_reg=CHUNK,
                            elem_size=128,
                            elem_step=256,
                            transpose=False,
                        )
                    gt.append(g)
            if VARIANT == "p2g1":
                outt0 = rep_pool.tile([CPC, 128], BF16)
                nc.scalar.copy(out=outt0, in_=gt[0][0:CPC, 0, :])
                nc.sync.dma_start(out=d_out[0:CPC, 0:128], in_=outt0)
                return

            # ---- pass 1: self-Gram over vocab ----
            if VARIANT in ("p2", "p2g", "p2n", "p2t"):
                scale_bc = sc.tile([128, CPC], F32)
                nc.vector.memset(scale_bc, 1.0)
            else:
                gram = p1ps.tile([W, W], F32, tag="gram")
                n_mm = NG * 4
                k = 0
                for i in range(NG // GL):
                    t = p1l.tile([128, GL, 4, 128], FP8)
                    nc.sync.dma_start(
                        out=t,
                        in_=d_p1t[i * GL * 128 : (i + 1) * GL * 128, :].rearrange(
                            "(g p) (j w) -> p g j w", p=128, j=4
                        ),
                    )
                    if VARIANT == "p1dma":
                        continue
                    for g in range(GL):
                        for j in range(4):
                            nc.tensor.matmul(
                                gram,
                                t[:, g, j, 0:W],
                                t[:, g, j, 0:W],
                                start=(k == 0),
                                stop=(k == n_mm - 1),
                            )
                            k += 1
                if VARIANT in ("p1mm", "p1dma"):
                    outt0 = rep_pool.tile([CPC, 8], BF16)
                    if VARIANT == "p1mm":
                        nc.scalar.copy(out=outt0, in_=gram[0:CPC, 0:8])
                    else:
                        nc.vector.memset(outt0, 0.0)
                    nc.sync.dma_start(out=d_out[0:CPC, 0:8], in_=outt0)
                    return

                # ---- sumsq (x65536) from Gram pieces ----
                gram_sb = sc.tile([W, W], F32, tag="gram_sb")
                nc.vector.tensor_copy(out=gram_sb, in_=gram)
                if VARIANT == "p1z":
                    outt0 = rep_pool.tile([CPC, 8], BF16)
                    nc.scalar.copy(out=outt0, in_=gram_sb[0:CPC, 0:8])
                    nc.sync.dma_start(out=d_out[0:CPC, 0:8], in_=outt0)
                    return
                dd = sc.tile([CPC, CPC], F32, tag="dd")
                nc.vector.tensor_tensor(
                    out=dd, in0=gram_sb[0:CPC, 0:CPC], in1=i96_sb, op=ALU.mult
                )
                t1 = sc.tile([CPC, 1], F32)
                nc.vector.reduce_sum(out=t1, in_=dd, axis=mybir.AxisListType.X)
                if VARIANT == "p1a":
                    _dummy_out(rep_pool, t1)
                    return
                # maT[16, 0:96] = M^T = A'E', maT[16, 96:112] = G = A'A'
                maT = sc.tile([R, W], F32, tag="maT")
                nc.sync.dma_start(out=maT, in_=gram_sb[CPC:W, :])
                # u = G @ sB + 2 M^T   [16, 96]
                u_ps = scps.tile([R, CPC], F32, tag="u")
                nc.tensor.matmul(u_ps, maT[:, CPC:W], sb_sb,
                                 start=True, stop=False)
                nc.tensor.matmul(u_ps, twoi_sb, maT[:, 0:CPC],
                                 start=False, stop=True)
                vsb = sc.tile([R, CPC], F32, tag="vsb")
                nc.vector.tensor_tensor(out=vsb, in0=u_ps, in1=sb_sb,
                                        op=ALU.mult)
                # ssT[1, 96] = ones^T @ vsb + t1^T
                red_ps = scps.tile([128, CPC], F32, tag="red")
                nc.tensor.matmul(red_ps[0:1, :], ones16_sb, vsb,
                                 start=True, stop=False)
                nc.tensor.matmul(red_ps[0:1, :], t1, idf_sb[:CPC, :CPC],
                                 start=False, stop=True, is_transpose=True)
                ssT = sc.tile([1, CPC], F32)
                nc.vector.tensor_copy(out=ssT, in_=red_ps[0:1, :])
                if VARIANT == "p1b":
                    outt0 = rep_pool.tile([CPC, 8], BF16)
                    nc.vector.memset(outt0, 0.0)
                    nc.vector.tensor_copy(out=outt0[0:1, 0:8], in_=ssT[:, 0:8])
                    nc.sync.dma_start(out=d_out[0:CPC, 0:8], in_=outt0)
                    return

                # ---- sclT = 256*mag * rsqrt(ssT)  [1, 96] ----
                nrm = sc.tile([1, CPC], F32)
                nc.scalar.activation(nrm, ssT, ACTF.Sqrt)
                nc.vector.tensor_scalar(
                    out=nrm, in0=nrm, scalar1=EF8 * 1e-8, scalar2=None,
                    op0=ALU.max,
                )
                r0 = sc.tile([1, CPC], F32)
                nc.vector.reciprocal(out=r0, in_=nrm)
                tn = sc.tile([1, CPC], F32)
                nc.vector.tensor_tensor(out=tn, in0=nrm, in1=r0, op=ALU.mult)
                nc.vector.tensor_scalar(
                    out=tn, in0=tn, scalar1=-1.0, scalar2=2.0,
                    op0=ALU.mult, op1=ALU.add,
                )
                r1 = sc.tile([1, CPC], F32)
                nc.vector.tensor_tensor(out=r1, in0=r0, in1=tn, op=ALU.mult)
                sclT = sc.tile([1, CPC], F32)
                nc.vector.tensor_tensor(out=sclT, in0=r1, in1=mag_sb,
                                        op=ALU.mult)

                # ---- broadcast to scale_bc [128, 96] ----
                sbc_ps = scps.tile([128, CPC], F32, tag="red")
                nc.tensor.matmul(sbc_ps, ones_sb, sclT, start=True, stop=True)
                scale_bc = sc.tile([128, CPC], F32)
                nc.vector.tensor_copy(out=scale_bc, in_=sbc_ps)

            if VARIANT == "p1":
                _dummy_out(rep_pool, scale_bc[0:CPC, 0:1])
                return
            if VARIANT == "p2g":
                _dummy_out(rep_pool)
                return

            # ---- pass 2: transpose A, A^T@sB + E, scale, store ----
            for c in range(nch):
                g = gt[c]
                tp_ps = p2tp.tile([R, NS, 128], BF16)
                for s in range(NS):
                    nc.tensor.transpose(tp_ps[:, s, :], g[:, s, CPC:W], idb_sb)
                astp = p2s.tile([R, NS, 128], BF16)
                nc.scalar.copy(out=astp, in_=tp_ps)
                if VARIANT == "p2t":
                    if c == 0:
                        outt0 = rep_pool.tile([CPC, 128], BF16)
                        nc.vector.memset(outt0, 0.0)
                        nc.vector.tensor_copy(out=outt0[0:R, :], in_=astp[:, 0, :])
                        nc.sync.dma_start(out=d_out[0:CPC, 0:128], in_=outt0)
                    continue
                for h in range(2):
                    ab_ps = p2ps.tile([128, HS, CPC], F32)
                    for q in range(HS):
                        s = h * HS + q
                        nc.tensor.matmul(
                            ab_ps[:, q, :], astp[:, s, :], sbbf_sb,
                            start=True, stop=False,
                        )
                        nc.tensor.matmul(
                            ab_ps[:, q, :], idb_sb, g[:, s, 0:CPC],
                            start=False, stop=True,
                        )
                    ot = p2o.tile([128, HS, CPC], BF16)
                    nc.vector.tensor_tensor(
                        out=ot, in0=ab_ps, in1=_bcast2(scale_bc, HS),
                        op=ALU.mult,
                    )
                    nc.sync.dma_start(
                        out=d_out[
                            c * 128 : (c + 1) * 128,
                            h * HS * CPC : (h + 1) * HS * CPC,
                        ].rearrange("p (s f) -> p s f", s=HS),
                        in_=ot,
                    )

        chunk_parity = _build.chunk_parity
        for _rep in range(NREP):
            if _rep:
                tc.strict_bb_all_engine_barrier()
            with (
                tc.tile_pool(name=f"rep{_rep}", bufs=1) as rep_pool,
                tc.tile_pool(name=f"p1l{_rep}", bufs=3) as p1l,
                tc.tile_pool(name=f"p1ps{_rep}", bufs=1, space="PSUM") as p1ps,
                tc.tile_pool(name=f"sc{_rep}", bufs=1) as sc,
                tc.tile_pool(name=f"scps{_rep}", bufs=1, space="PSUM") as scps,
                tc.tile_pool(name=f"p2g{_rep}", bufs=1) as p2g,
                tc.tile_pool(name=f"p2tp{_rep}", bufs=2, space="PSUM") as p2tp,
                tc.tile_pool(name=f"p2s{_rep}", bufs=2) as p2s,
                tc.tile_pool(name=f"p2ps{_rep}", bufs=3, space="PSUM") as p2ps,
                tc.tile_pool(name=f"p2o{_rep}", bufs=3) as p2o,
            ):
                _emit(chunk_parity, rep_pool, p1l, p1ps, sc, scps, p2g, p2tp,
                      p2s, p2ps, p2o)

    nc.compile()
    _CACHED[key] = nc
    return nc


_build.chunk_parity = []


def _host_prep(inputs, embeddings, lora_a, lora_b, magnitude):
    E = np.asarray(embeddings, np.float32)
    A = np.asarray(lora_a, np.float32)
    B = np.asarray(lora_b, np.float32)
    mag = np.asarray(magnitude, np.float32)
    ids = np.asarray(inputs).astype(np.int64).reshape(-1)

    # ---- token parity bucketing ----
    even_pos = np.flatnonzero((ids & 1) == 0)
    odd_pos = np.flatnonzero((ids & 1) == 1)
    ne, no = len(even_pos), len(odd_pos)
    nce = (ne + CHUNK - 1) // CHUNK
    nco = (no + CHUNK - 1) // CHUNK
    nch = nce + nco
    perm = np.concatenate([even_pos, odd_pos])
    chunk_parity = [0] * nce + [1] * nco
    pair_ids = np.zeros(nch * CHUNK, np.int64)
    pair_ids[:ne] = ids[even_pos] // 2
    pair_ids[nce * CHUNK : nce * CHUNK + no] = ids[odd_pos] // 2
    pidx_np = np.tile(
        pair_ids.astype(np.int16).reshape(nch * CHUNK // 16, 16).T, (8, 1)
    ).copy()

    # ---- pass-1 fp8 table: [E*256 | A*256 | pad] vocab-major ----
    fp8 = ml_dtypes.float8_e4m3
    EAf = np.zeros((VP, 128), np.float32)
    EAf[:V, CPC : CPC + R] = EF8 * A

    # ---- pass-2 bf16 pair table ----
    Epad = np.zeros((2 * PAIRS, D), np.float32)
    Epad[:V] = E
    Apad = np.zeros((2 * PAIRS, R), np.float32)
    Apad[:V] = A

    idf_np = np.eye(128, dtype=np.float32)
    idb_np = np.eye(128, dtype=ml_dtypes.bfloat16)
    i96_np = np.eye(CPC, dtype=np.float32)
    twoi_np = 2.0 * np.eye(R, dtype=np.float32)
    ones16_np = np.ones((R, 1), np.float32)
    ones1_np = np.ones((1, 128), np.float32)

    in_maps = []
    for c in range(N_CORES):
        cols = slice(CPC * c, CPC * (c + 1))
        EAf[:V, 0:CPC] = EF8 * E[:, cols]
        p1t = EAf.astype(fp8).reshape(VP // 4, 512)

        ec = Epad[:, cols]
        gtab = np.zeros((PAIRS, 256), dtype=ml_dtypes.bfloat16)
        gtab[:, 0:CPC] = ec[0::2]
        gtab[:, CPC : CPC + R] = Apad[0::2]
        gtab[:, 128 : 128 + CPC] = ec[1::2]
        gtab[:, 128 + CPC : 128 + CPC + R] = Apad[1::2]

        sb = SCALING * B[:, cols]                      # [R, CPC]
        in_maps.append(
            {
                "p1t": p1t,
                "gtab": gtab,
                "sb": np.ascontiguousarray(sb),
                "sbbf": sb.astype(ml_dtypes.bfloat16),
                "magT": np.ascontiguousarray(EF8 * mag[cols])[None, :],
                "identf": idf_np,
                "idb": idb_np,
                "i96": i96_np,
                "twoi16": twoi_np,
                "ones16": ones16_np,
                "ones1": ones1_np,
                "pidx": pidx_np,
            }
        )
    return in_maps, perm, ne, no, nce, nch, chunk_parity


def kernel(inputs, embeddings, lora_a, lora_b, magnitude, _trace=False):
    in_maps, perm, ne, no, nce, nch, chunk_parity = _host_prep(
        inputs, embeddings, lora_a, lora_b, magnitude
    )
    _build.chunk_parity = chunk_parity
    nc = _build(nch)
    res = bass_utils.run_bass_kernel_spmd(
        nc, in_maps, core_ids=list(range(N_CORES)), trace=_trace
    )
    shp = np.asarray(inputs).shape
    out = np.empty((NTOK, D), np.float32)
    valid = np.concatenate([np.arange(ne), nce * CHUNK + np.arange(no)])
    for c in range(N_CORES):
        blk = np.asarray(res.results[c]["out"], dtype=np.float32)
        # [nch*128, 8*96] -> token-ordered [nch*1024, 96]
        blk = blk.reshape(nch, 128, NS, CPC).transpose(0, 2, 1, 3)
        blk = blk.reshape(nch * CHUNK, CPC)
        out[perm, CPC * c : CPC * (c + 1)] = blk[valid]
    out = out.reshape(shp + (D,))
    if _trace:
        return out, res
    return out



# revision 6
# speedup vs baseline: 5.1109x; 5.1109x over previous
"""DoRA embedding kernel for 8 Trainium2 NeuronCores.

Math (reference):
    C = E + s * A @ B                  # [V, D]
    n = max(||C||_col, 1e-8)           # [D]
    out = (C / n * mag)[token_ids]     # [B, S, D]

Strategy: shard D=768 columns across 8 cores (96 cols each), no collectives.

Pass 1 (norms) — self-Gram on PE, fp8:
    Stream vocab-major tiles T = [256*E_cols | 256*A] (fp8, [128v, 112], four
    vocab rows packed per 512B table row for full-rate DMA) through
    matmul(T^T T), accumulating G = [[E'E', E'A'],[A'E', A'A']] in one PSUM
    bank (396 matmuls, one accumulation group).  Then
        sumsq*65536 = diag(E'E') + sum_r (sB (.) (2M^T + A'A' sB))
    via small matmuls operating directly on Gram slices at partitions 96:112
    (tile_position row offset 96 — no SBUF repack DMA), rsqrt via DVE
    reciprocal + one Newton step, giving sclT = mag/||C||_col in [1, 96].

Pass 2 (lookup) — TRANSPOSED parity-bucketed gathers + one fused matmul:
    dma_gather(transpose=True) lands each 1024-token chunk feature-major:
    g[p, 0, t] = row-element p of token t, i.e. partitions 0:96 = E_c^T,
    96:112 = A^T.  The entire (E + A sB) * s is then ONE matmul per
    512-token block with the fused stationary W2 = [[diag(s)], [s (.) sB]]
    ([112, 96] bf16, built on device from sclT):
        psum[col, tok] = sum_k W2[k, col] g[k, tok]
                       = s_col E^T[col, tok] + sum_r s_col sB[r, col] A^T[r, tok].
    DVE/ACT alternate evacuating psum -> bf16 [96, 1024] tiles, DMA out
    column-major [96, nch*1024]; host transposes/un-permutes/upcasts.
"""

import sys
from contextlib import ExitStack

import numpy as np

for _p in ("/opt/trn_rl_repo",):
    if _p not in sys.path:
        sys.path.append(_p)

import ml_dtypes
import concourse.bass as bass
import concourse.bacc as bacc
import concourse.tile as tile
from concourse import mybir, bass_utils

F32 = mybir.dt.float32
BF16 = mybir.dt.bfloat16
FP8 = mybir.dt.float8e4
I16 = mybir.dt.int16
ALU = mybir.AluOpType
ACTF = mybir.ActivationFunctionType

V, D, R = 50257, 768, 16
SCALING = 32.0 / 16.0
N_CORES = 8
CPC = D // N_CORES          # columns per core = 96
W = CPC + R                 # 112 packed feature width
EF8 = 256.0                 # fp8 pre-scale for E and A in pass 1
VP = 50688                  # vocab padded: 99 groups of 512 rows
NG = VP // 512              # 99 row-groups (4 matmuls each)
GL = 9                      # groups per DMA macro-load (99 = 11 * 9)
PAIRS = ((V + 1) // 2 + 127) // 128 * 128  # 25216 pair rows
NTOK = 8 * 2048
CHUNK = 1024                # tokens per dma_gather (num_idxs>1024 crashes Q7)
HB = 512                    # tokens per matmul block (one PSUM bank)


def _apply_drain_patch():
    """walrus in this container rejects >1 sem-wait on the Tile tail drain
    ("Too many sync wait commands"); split the waits across chained drains."""
    import concourse.tile as _tile_mod
    if getattr(_tile_mod.TileContext, "_drain_patch_applied", False):
        return

    def _drain_and_barrier(self, tick_clock, wait_clock):
        from concourse.tile import ScopedClock

        nc = self.nc
        drain_inst = nc.sync.drain()
        wait_clock.add_sem_waits(
            drain_inst.ins, ScopedClock({None: tick_clock.global_clock})
        )
        si = drain_inst.ins.sync_info
        if si is not None and si.on_wait and len(si.on_wait) > 1:
            waits = list(si.on_wait)
            del si.on_wait[1:]
            for w in waits[1:]:
                extra = nc.sync.drain()
                esi = extra.ins.sync_info
                if esi is None:
                    extra.ins.sync_info = mybir.SyncInfo(on_wait=[w], on_update=[])
                else:
                    esi.on_wait.append(w)
        nc.all_engine_barrier()
        assert self.sems is not None
        popped = nc._tile_sem_poison_stack.pop()
        assert popped is self._sem_poison
        nc.clear_and_free_semaphores(list(self.sems.allocated().values()))
        nc.all_engine_barrier()

    _tile_mod.TileContext._drain_and_barrier = _drain_and_barrier
    _tile_mod.TileContext._drain_patch_applied = True


_CACHED = {}
NREP = 1
VARIANT = "full"


def _build(nch):
    key = (NREP, VARIANT, nch, tuple(_build.chunk_parity))
    if key in _CACHED:
        return _CACHED[key]
    _apply_drain_patch()

    nc = bacc.Bacc("TRN2", target_bir_lowering=False, debug=False)
    d_p1t = nc.dram_tensor("p1t", [VP // 4, 512], FP8, kind="ExternalInput").ap()
    d_gtab = nc.dram_tensor("gtab", [PAIRS, 256], BF16, kind="ExternalInput").ap()
    # sb96: rows 96:112 hold SCALING*B[:, cols] (f32); others zero
    d_sb96 = nc.dram_tensor("sb96", [128, CPC], F32, kind="ExternalInput").ap()
    d_mag = nc.dram_tensor("magT", [1, CPC], F32, kind="ExternalInput").ap()
    d_idf = nc.dram_tensor("identf", [128, 128], F32, kind="ExternalInput").ap()
    d_i96 = nc.dram_tensor("i96", [CPC, CPC], F32, kind="ExternalInput").ap()
    # twoi96: rows 96:112 hold 2*I_16; others zero
    d_twoi = nc.dram_tensor("twoi96", [128, R], F32, kind="ExternalInput").ap()
    # ones16_96: rows 96:112 hold 1.0; others zero
    d_ones16 = nc.dram_tensor("ones16_96", [128, 1], F32, kind="ExternalInput").ap()
    d_ones = nc.dram_tensor("ones1", [1, 128], F32, kind="ExternalInput").ap()
    d_pidx = nc.dram_tensor("pidx", [128, nch * (CHUNK // 16)], I16,
                            kind="ExternalInput").ap()
    # column-major output: token j = c*1024 + t at column j, rows = 96 cols
    d_out = nc.dram_tensor("out", [CPC, nch * CHUNK], BF16,
                           kind="ExternalOutput").ap()

    with tile.TileContext(nc) as tc, ExitStack() as ctx:
        const = ctx.enter_context(tc.tile_pool(name="const", bufs=1))

        sb96_sb = const.tile([128, CPC], F32)
        nc.sync.dma_start(out=sb96_sb, in_=d_sb96)
        mag_sb = const.tile([1, CPC], F32)
        nc.sync.dma_start(out=mag_sb, in_=d_mag)
        idf_sb = const.tile([128, 128], F32)
        nc.sync.dma_start(out=idf_sb, in_=d_idf)
        i96_sb = const.tile([CPC, CPC], F32)
        nc.sync.dma_start(out=i96_sb, in_=d_i96)
        twoi_sb = const.tile([128, R], F32)
        nc.sync.dma_start(out=twoi_sb, in_=d_twoi)
        ones16_sb = const.tile([128, 1], F32)
        nc.sync.dma_start(out=ones16_sb, in_=d_ones16)
        ones_sb = const.tile([1, 128], F32)
        nc.sync.dma_start(out=ones_sb, in_=d_ones)
        pidx_sb = const.tile([128, nch * (CHUNK // 16)], I16)
        nc.sync.dma_start(out=pidx_sb, in_=d_pidx)

    # number of evacuations routed to ACT vs DVE (interleave by index)

        def _dummy_out(rep_pool, src=None):
            outt0 = rep_pool.tile([CPC, 8], BF16)
            nc.vector.memset(outt0, 0.0)
            if src is not None:
                nc.vector.tensor_copy(out=outt0[0:1, 0:1], in_=src)
            nc.sync.dma_start(out=d_out[0:CPC, 0:8], in_=outt0)

        def _emit(chunk_parity, rep_pool, p1l, p1ps, sc, scps, p2g, p2ps, p2o):
            if VARIANT == "nop":
                _dummy_out(rep_pool)
                return

            # ---- pass 2 gathers: transposed 256B rows (issue first) ----
            import os
            gn = int(os.environ.get("GATHER_N", str(CHUNK)))
            # single_packet=True overflows the 64-desc/packet ring limit at
            # num_idxs=1024 on the transpose path (device-fatal); keep False.
            gsp = os.environ.get("GATHER_SP", "0") == "1"
            gt = []
            n_gather = {"p2g1": 1}.get(VARIANT, nch)
            if VARIANT not in ("p1", "p1mm", "p1dma"):
                for c in range(n_gather):
                    g = p2g.tile([128, 1, CHUNK], BF16)
                    off = 128 * chunk_parity[c]
                    nc.gpsimd.dma_gather(
                        g[:, :, 0:gn],
                        d_gtab[:, off : off + 128],
                        pidx_sb[:, c * (CHUNK // 16) : (c + 1) * (CHUNK // 16)],
                        num_idxs=gn,
                        num_idxs_reg=gn,
                        elem_size=128,
                        elem_step=256,
                        transpose=True,
                        single_packet=gsp,
                    )
                    gt.append(g)
            if VARIANT in ("p2g", "p2g1"):
                outt0 = rep_pool.tile([CPC, 128], BF16)
                nc.scalar.copy(out=outt0, in_=gt[0][0:CPC, 0, 0:128])
                nc.sync.dma_start(out=d_out[0:CPC, 0:128], in_=outt0)
                return

            # ---- pass 1: self-Gram over vocab ----
            if VARIANT == "p2":
                w2 = sc.tile([W, CPC], BF16, tag="w2")
                nc.vector.memset(w2, 0.0)
                nc.vector.memset(w2[0:CPC, :], 1.0)
            else:
                gram = p1ps.tile([W, W], F32, tag="gram")
                n_mm = NG * 4
                k = 0
                for i in range(NG // GL):
                    t = p1l.tile([128, GL, 4, 128], FP8)
                    nc.sync.dma_start(
                        out=t,
                        in_=d_p1t[i * GL * 128 : (i + 1) * GL * 128, :].rearrange(
                            "(g p) (j w) -> p g j w", p=128, j=4
                        ),
                    )
                    if VARIANT == "p1dma":
                        continue
                    for g in range(GL):
                        for j in range(4):
                            nc.tensor.matmul(
                                gram,
                                t[:, g, j, 0:W],
                                t[:, g, j, 0:W],
                                start=(k == 0),
                                stop=(k == n_mm - 1),
                            )
                            k += 1
                if VARIANT in ("p1mm", "p1dma"):
                    outt0 = rep_pool.tile([CPC, 8], BF16)
                    if VARIANT == "p1mm":
                        nc.scalar.copy(out=outt0, in_=gram[0:CPC, 0:8])
                    else:
                        nc.vector.memset(outt0, 0.0)
                    nc.sync.dma_start(out=d_out[0:CPC, 0:8], in_=outt0)
                    return

                # ---- sumsq (x65536) from Gram pieces ----
                gram_sb = sc.tile([W, W], F32, tag="gram_sb")
                nc.vector.tensor_copy(out=gram_sb, in_=gram)
                # t1[96, 1] = diag(E'E')
                dd = sc.tile([CPC, CPC], F32, tag="dd")
                nc.vector.tensor_tensor(
                    out=dd, in0=gram_sb[0:CPC, 0:CPC], in1=i96_sb, op=ALU.mult
                )
                t1 = sc.tile([CPC, 1], F32)
                nc.vector.reduce_sum(out=t1, in_=dd, axis=mybir.AxisListType.X)
                # u[16@96, 96] = A'A' @ sB + 2I @ M^T  (Gram slices in place)
                u_ps = scps.tile([128, CPC], F32, tag="u")
                nc.tensor.matmul(u_ps[96:W, :], gram_sb[CPC:W, CPC:W],
                                 sb96_sb[96:W, :], start=True, stop=False,
                                 tile_position=(96, 96))
                nc.tensor.matmul(u_ps[96:W, :], twoi_sb[96:W, :],
                                 gram_sb[CPC:W, 0:CPC], start=False, stop=True,
                                 tile_position=(96, 96))
                vsb = sc.tile([128, CPC], F32, tag="vsb")
                nc.vector.tensor_tensor(out=vsb[96:W, :], in0=u_ps[96:W, :],
                                        in1=sb96_sb[96:W, :], op=ALU.mult)
                # ssT[1, 96] = ones16^T @ vsb + t1^T
                red_ps = scps.tile([128, CPC], F32, tag="red")
                nc.tensor.matmul(red_ps[0:1, :], ones16_sb[96:W, :],
                                 vsb[96:W, :], start=True, stop=False,
                                 tile_position=(96, 0))
                nc.tensor.matmul(red_ps[0:1, :], t1, idf_sb[:CPC, :CPC],
                                 start=False, stop=True, is_transpose=True)
                ssT = sc.tile([1, CPC], F32)
                nc.vector.tensor_copy(out=ssT, in_=red_ps[0:1, :])

                # ---- sclT = 256*mag * rsqrt(ssT)  [1, 96] ----
                nrm = sc.tile([1, CPC], F32)
                nc.scalar.activation(nrm, ssT, ACTF.Sqrt)
                nc.vector.tensor_scalar(
                    out=nrm, in0=nrm, scalar1=EF8 * 1e-8, scalar2=None,
                    op0=ALU.max,
                )
                r0 = sc.tile([1, CPC], F32)
                nc.vector.reciprocal(out=r0, in_=nrm)
                tn = sc.tile([1, CPC], F32)
                nc.vector.tensor_tensor(out=tn, in0=nrm, in1=r0, op=ALU.mult)
                nc.vector.tensor_scalar(
                    out=tn, in0=tn, scalar1=-1.0, scalar2=2.0,
                    op0=ALU.mult, op1=ALU.add,
                )
                r1 = sc.tile([1, CPC], F32)
                nc.vector.tensor_tensor(out=r1, in0=r0, in1=tn, op=ALU.mult)
                sclT = sc.tile([1, CPC], F32)
                nc.vector.tensor_tensor(out=sclT, in0=r1, in1=mag_sb,
                                        op=ALU.mult)

                # ---- W2 [112, 96] bf16 = [[diag(s)], [s (.) sB]] ----
                srep_ps = scps.tile([128, CPC], F32, tag="srep")
                nc.tensor.matmul(srep_ps[0:CPC, :], ones_sb[0:1, 0:CPC], sclT,
                                 start=True, stop=True)
                nc.tensor.matmul(srep_ps[96:W, :], ones_sb[0:1, 0:R], sclT,
                                 start=True, stop=True, tile_position=(0, 96))
                w2 = sc.tile([W, CPC], BF16, tag="w2")
                nc.vector.tensor_tensor(out=w2[0:CPC, :], in0=i96_sb,
                                        in1=srep_ps[0:CPC, :], op=ALU.mult)
                nc.vector.tensor_tensor(out=w2[96:W, :], in0=sb96_sb[96:W, :],
                                        in1=srep_ps[96:W, :], op=ALU.mult)

            if VARIANT == "p1":
                _dummy_out(rep_pool, sclT[0:1, 0:1])
                return

            # ---- pass 2: one fused matmul per 512-token block ----
            for c in range(nch):
                g = gt[c]
                ot = p2o.tile([CPC, 2, HB], BF16)
                for h in range(2):
                    ps = p2ps.tile([128, HB], F32, tag="ps")
                    nc.tensor.matmul(
                        ps[0:CPC, :], w2, g[0:W, 0, h * HB : (h + 1) * HB],
                        start=True, stop=True,
                    )
                    if (2 * c + h) % 2 == 0:
                        nc.vector.tensor_copy(out=ot[:, h, :], in_=ps[0:CPC, :])
                    else:
                        nc.scalar.copy(out=ot[:, h, :], in_=ps[0:CPC, :])
                nc.sync.dma_start(
                    out=d_out[:, c * CHUNK : (c + 1) * CHUNK].rearrange(
                        "p (h f) -> p h f", h=2
                    ),
                    in_=ot,
                )

        chunk_parity = _build.chunk_parity
        for _rep in range(NREP):
            if _rep:
                tc.strict_bb_all_engine_barrier()
            with (
                tc.tile_pool(name=f"rep{_rep}", bufs=1) as rep_pool,
                tc.tile_pool(name=f"p1l{_rep}", bufs=3) as p1l,
                tc.tile_pool(name=f"p1ps{_rep}", bufs=1, space="PSUM") as p1ps,
                tc.tile_pool(name=f"sc{_rep}", bufs=1) as sc,
                tc.tile_pool(name=f"scps{_rep}", bufs=1, space="PSUM") as scps,
                tc.tile_pool(name=f"p2g{_rep}", bufs=nch) as p2g,
                tc.tile_pool(name=f"p2ps{_rep}", bufs=4, space="PSUM") as p2ps,
                tc.tile_pool(name=f"p2o{_rep}", bufs=3) as p2o,
            ):
                _emit(chunk_parity, rep_pool, p1l, p1ps, sc, scps, p2g,
                      p2ps, p2o)

    nc.compile()
    _CACHED[key] = nc
    return nc


_build.chunk_parity = []


def _host_prep(inputs, embeddings, lora_a, lora_b, magnitude):
    E = np.asarray(embeddings, np.float32)
    A = np.asarray(lora_a, np.float32)
    B = np.asarray(lora_b, np.float32)
    mag = np.asarray(magnitude, np.float32)
    ids = np.asarray(inputs).astype(np.int64).reshape(-1)

    # ---- token parity bucketing ----
    even_pos = np.flatnonzero((ids & 1) == 0)
    odd_pos = np.flatnonzero((ids & 1) == 1)
    ne, no = len(even_pos), len(odd_pos)
    nce = (ne + CHUNK - 1) // CHUNK
    nco = (no + CHUNK - 1) // CHUNK
    nch = nce + nco
    perm = np.concatenate([even_pos, odd_pos])
    chunk_parity = [0] * nce + [1] * nco
    pair_ids = np.zeros(nch * CHUNK, np.int64)
    pair_ids[:ne] = ids[even_pos] // 2
    pair_ids[nce * CHUNK : nce * CHUNK + no] = ids[odd_pos] // 2
    pidx_np = np.tile(
        pair_ids.astype(np.int16).reshape(nch * CHUNK // 16, 16).T, (8, 1)
    ).copy()

    # ---- pass-1 fp8 table: [E*256 | A*256 | pad] vocab-major ----
    fp8 = ml_dtypes.float8_e4m3
    EAf = np.zeros((VP, 128), np.float32)
    EAf[:V, CPC : CPC + R] = EF8 * A

    # ---- pass-2 bf16 pair table ----
    Epad = np.zeros((2 * PAIRS, D), np.float32)
    Epad[:V] = E
    Apad = np.zeros((2 * PAIRS, R), np.float32)
    Apad[:V] = A

    idf_np = np.eye(128, dtype=np.float32)
    i96_np = np.eye(CPC, dtype=np.float32)
    twoi_np = np.zeros((128, R), np.float32)
    twoi_np[96:W, :] = 2.0 * np.eye(R, dtype=np.float32)
    ones16_np = np.zeros((128, 1), np.float32)
    ones16_np[96:W, :] = 1.0
    ones1_np = np.ones((1, 128), np.float32)

    in_maps = []
    for c in range(N_CORES):
        cols = slice(CPC * c, CPC * (c + 1))
        EAf[:V, 0:CPC] = EF8 * E[:, cols]
        p1t = EAf.astype(fp8).reshape(VP // 4, 512)

        ec = Epad[:, cols]
        gtab = np.zeros((PAIRS, 256), dtype=ml_dtypes.bfloat16)
        gtab[:, 0:CPC] = ec[0::2]
        gtab[:, CPC : CPC + R] = Apad[0::2]
        gtab[:, 128 : 128 + CPC] = ec[1::2]
        gtab[:, 128 + CPC : 128 + CPC + R] = Apad[1::2]

        sb96 = np.zeros((128, CPC), np.float32)
        sb96[96:W, :] = SCALING * B[:, cols]
        in_maps.append(
            {
                "p1t": p1t,
                "gtab": gtab,
                "sb96": sb96,
                "magT": np.ascontiguousarray(EF8 * mag[cols])[None, :],
                "identf": idf_np,
                "i96": i96_np,
                "twoi96": twoi_np,
                "ones16_96": ones16_np,
                "ones1": ones1_np,
                "pidx": pidx_np,
            }
        )
    return in_maps, perm, ne, no, nce, nch, chunk_parity


def kernel(inputs, embeddings, lora_a, lora_b, magnitude, _trace=False):
    in_maps, perm, ne, no, nce, nch, chunk_parity = _host_prep(
        inputs, embeddings, lora_a, lora_b, magnitude
    )
    _build.chunk_parity = chunk_parity
    nc = _build(nch)
    res = bass_utils.run_bass_kernel_spmd(
        nc, in_maps, core_ids=list(range(N_CORES)), trace=_trace
    )
    shp = np.asarray(inputs).shape
    out = np.empty((NTOK, D), np.float32)
    valid = np.concatenate([np.arange(ne), nce * CHUNK + np.arange(no)])
    for c in range(N_CORES):
        blk = np.asarray(res.results[c]["out"], dtype=np.float32)
        # [96, nch*1024] column-major -> token-ordered [nch*1024, 96]
        out[perm, CPC * c : CPC * (c + 1)] = blk[:, valid].T
    out = out.reshape(shp + (D,))
    if _trace:
        return out, res
    return out
